# revision 2
# baseline (speedup 1.0000x reference)
"""DecoderRNNTAtt fused Trainium2 kernel v2 - self-contained.

Strategy vs v1: gates computed TRANSPOSED ([128 units, 8 batch] psum tiles,
matmul cost on TRN2's cost model scales with out free size, so free=8 makes
the whole recurrence ~free), attention tanh bias pre-added on the PE into
PSUM (one big tanh per [128,512] tile instead of 32 biased [128,128] ACTs),
joint z written straight from PSUM to DRAM (b_jo==0), and the per-step
h0/h1 exchange is either a small bf16 AllGather (FABRIC="ag") or direct
SBUF->SBUF remote DMA broadcasts with an XOR slot permutation folded into
per-core weight layouts (FABRIC="rdma").
"""

import numpy as np
import ml_dtypes

import concourse.bass as bass
import concourse.mybir as mybir
import concourse.tile as tile
from concourse.tile import add_dep_helper
from concourse import bacc

BF = ml_dtypes.bfloat16
F32 = mybir.dt.float32
BF16 = mybir.dt.bfloat16
I32 = mybir.dt.int32

B, T, U = 8, 128, 32
EPROJS, DUNITS, EMB, ATT, JOINT, ODIM = 1024, 1024, 512, 512, 1024, 4096
NCORE = 8
KE = EPROJS // 128
KD = DUNITS // 128
KJ = JOINT // 128
KA = ATT // 128
KEM = EMB // 128
GS = 512                   # per-core gate slice (4 gates x 128 units)
AF = mybir.ActivationFunctionType
ALU = mybir.AluOpType

FABRIC = "ag"              # "ag" or "rdma"
REPEAT = 1
NEX = U + 1                # exchanges per rep


def host_prep(inputs, fabric=None):
    fabric = fabric or FABRIC
    f32 = np.float32
    hs = np.asarray(inputs["hs_pad"], f32)
    ys = np.asarray(inputs["ys_in_pad"], np.int32)
    embed = np.asarray(inputs["embed"], f32)
    W_ih0 = np.asarray(inputs["W_ih0"], f32)
    W_hh0 = np.asarray(inputs["W_hh0"], f32)
    W_ih1 = np.asarray(inputs["W_ih1"], f32)
    W_hh1 = np.asarray(inputs["W_hh1"], f32)
    w_dec = np.asarray(inputs["w_dec"], f32)
    w_jd = np.asarray(inputs["w_jd"], f32)
    hl = np.asarray(inputs["hlens"], np.int32)
    assert np.all(np.asarray(inputs["b_jo"], f32) == 0.0), "b_jo must be zero"

    # hsT[k, p, b*T+t] = hs[b, t, 128k+p]
    hsT = np.ascontiguousarray(hs.transpose(2, 0, 1).reshape(KE, 128, B * T)).astype(BF)

    # host embedding gather: eyT[e*128+p, u*8+b] = embed[ys[b,u], e*128+p]
    eys = embed[ys]                                  # [B, U, EMB]
    eyT = np.ascontiguousarray(
        eys.transpose(2, 1, 0).reshape(EMB, U * B)).astype(BF)

    # mask[b, t] = -1e9 where t >= max(hlens,1)
    maskb = np.where(np.arange(T)[None, :] < np.maximum(hl, 1)[:, None],
                     0.0, -1.0e9).astype(f32)

    # q-outer block-diag ones: ones8[b, h*512 + b2*128 + t] = (b == h*4+b2)
    ones8 = np.zeros((B, 2 * 512), f32)
    for b in range(B):
        h, b2 = divmod(b, 4)
        ones8[b, h * 512 + b2 * 128:(h * 512 + (b2 + 1) * 128)] = 1.0

    shared = {
        "hsT": hsT,
        "eyT": eyT,
        "maskb": maskb,
        "ones8": ones8.astype(BF),
        "w_enc_bf": np.asarray(inputs["w_enc"], f32).astype(BF),
        "w_je_bf": np.asarray(inputs["w_je"], f32).astype(BF),
        "w_jo_bf": np.asarray(inputs["w_jo"], f32).astype(BF),
        "v_att_r": np.ascontiguousarray(
            np.asarray(inputs["v_att"], f32).reshape(KA, 128).T).astype(BF),
        "b_att_r": np.ascontiguousarray(
            np.asarray(inputs["b_att"], f32).reshape(KA, 128).T),
        "b_je_r": np.ascontiguousarray(
            np.asarray(inputs["b_je"], f32).reshape(KJ, 128).T),
        "idbf": np.eye(128, dtype=f32).astype(BF),
        "id8": np.eye(8, dtype=f32),
    }

    per_core = []
    for c in range(NCORE):
        perm = [c ^ j for j in range(NCORE)] if fabric == "rdma" \
            else list(range(NCORE))
        punits = np.concatenate([p * 128 + np.arange(128) for p in perm])
        idx = np.concatenate([g * DUNITS + c * 128 + np.arange(128)
                              for g in range(4)])
        selb = np.zeros((128, NCORE * B), f32)
        for j in range(NCORE):
            selb[:, j * B + c] = 1.0
        pc = {
            "hsTb": np.ascontiguousarray(hsT[:, :, c * T:(c + 1) * T]),
            "selb": selb.astype(BF),
            "WeT": np.ascontiguousarray(W_ih0[idx, :EMB].T).astype(BF),
            "WaT": np.ascontiguousarray(W_ih0[idx, EMB:].T).astype(BF),
            "Whh0T": np.ascontiguousarray(W_hh0[idx][:, punits].T).astype(BF),
            "Wih1T": np.ascontiguousarray(W_ih1[idx][:, punits].T).astype(BF),
            "Whh1T": np.ascontiguousarray(W_hh1[idx][:, punits].T).astype(BF),
            "wdecT": np.ascontiguousarray(w_dec[punits, :]).astype(BF),
            "wjdT": np.ascontiguousarray(w_jd[punits, :]).astype(BF),
            "b0_r": np.ascontiguousarray(
                (np.asarray(inputs["b0"], f32)[idx].reshape(4, 128)
                 * np.array([[0.5], [0.5], [1.0], [0.5]], f32)).T),
            "b1_r": np.ascontiguousarray(
                (np.asarray(inputs["b1"], f32)[idx].reshape(4, 128)
                 * np.array([[0.5], [0.5], [1.0], [0.5]], f32)).T),
        }
        per_core.append({**shared, **pc})
    return per_core


def build_nc(fabric=None, repeat=None):
    fabric = fabric or FABRIC
    repeat = repeat or REPEAT
    nc = bacc.Bacc("TRN2", target_bir_lowering=False, debug=False,
                   num_devices=NCORE)

    def din(name, shape, dt):
        return nc.dram_tensor(name, shape, dt, kind="ExternalInput").ap()

    D = dict(
        hsT=din("hsT", [KE, 128, B * T], BF16),
        hsTb=din("hsTb", [KE, 128, T], BF16),
        eyT=din("eyT", [KEM * 128, U * B], BF16),
        maskb=din("maskb", [B, T], F32),
        ones8=din("ones8", [B, 2 * 512], BF16),
        wenc=din("w_enc_bf", [EPROJS, ATT], BF16),
        wje=din("w_je_bf", [EPROJS, JOINT], BF16),
        wjo=din("w_jo_bf", [JOINT, ODIM], BF16),
        vatt=din("v_att_r", [128, KA], BF16),
        batt=din("b_att_r", [128, KA], F32),
        bje=din("b_je_r", [128, KJ], F32),
        idbf=din("idbf", [128, 128], BF16),
        id8=din("id8", [8, 8], F32),
        selb=din("selb", [128, NCORE * B], BF16),
        WeT=din("WeT", [EMB, GS], BF16),
        WaT=din("WaT", [EPROJS, GS], BF16),
        Whh0T=din("Whh0T", [DUNITS, GS], BF16),
        Wih1T=din("Wih1T", [DUNITS, GS], BF16),
        Whh1T=din("Whh1T", [DUNITS, GS], BF16),
        wdecT=din("wdecT", [DUNITS, ATT], BF16),
        wjdT=din("wjdT", [DUNITS, JOINT], BF16),
        b0_r=din("b0_r", [128, 4], F32),
        b1_r=din("b1_r", [128, 4], F32),
        z=nc.dram_tensor("z", [128, U * ODIM], BF16, kind="ExternalOutput").ap(),
    )

    sems = {}
    if fabric == "rdma":
        sems["rsem"] = nc.alloc_semaphore("rx_sem")
        sems["lsem"] = nc.alloc_semaphore("tx_sem")
        sems["psem"] = nc.alloc_semaphore("prep_sem")

    post_waits = []
    with tile.TileContext(nc) as tc:
        _emit(nc, tc, D, fabric, repeat, sems, post_waits)
    for instr, sem, val in post_waits:
        instr._wait_ge(sem, val)
    nc.compile()
    return nc


def _emit(nc, tc, D, fabric, repeat, sems, post_waits):
    with (
        tc.tile_pool(name="res", bufs=1) as res,
        tc.tile_pool(name="ps_big", bufs=3, space="PSUM") as ps_big,
        tc.tile_pool(name="ps_g", bufs=2, space="PSUM") as ps_g,
        tc.tile_pool(name="ps_q", bufs=1, space="PSUM") as ps_q,
        tc.tile_pool(name="ps_sm", bufs=2, space="PSUM") as ps_sm,
        tc.tile_pool(name="dram", bufs=1, space="DRAM") as dram,
    ):
        # ---------------- resident tiles ----------------
        r = {}
        r["preT"] = [res.tile([128, B * 128], BF16, tag=f"preT{a}", name=f"preT{a}")
                     for a in range(KA)]
        r["P_sb"] = [res.tile([128, GS], BF16, tag=f"P{b}", name=f"P{b}")
                     for b in range(B)]
        r["jeT"] = [res.tile([128, 128], BF16, tag=f"jeT{j}", name=f"jeT{j}")
                    for j in range(KJ)]
        r["eyT"] = [res.tile([128, U * B], BF16, tag=f"eyT{e}", name=f"eyT{e}")
                    for e in range(KEM)]
        for nm, cols in [("Whh0T", KD * GS), ("Wih1T", KD * GS),
                         ("Whh1T", KD * GS), ("wdecT", KD * ATT),
                         ("wjdT", KD * JOINT), ("wjo", KJ * ODIM),
                         ("WeT", KEM * GS)]:
            r[nm] = res.tile([128, cols], BF16, tag=nm, name=nm)
        for nm, cols, dt in [("vatt", KA, BF16), ("batt", KA, F32),
                             ("bje", KJ, F32), ("idbf", 128, BF16),
                             ("maskb", T, F32), ("b0_r", 4, F32),
                             ("b1_r", 4, F32)]:
            r[nm] = res.tile([128, cols] if nm != "maskb" else [B, T], dt,
                             tag=nm, name=nm)
        r["id8"] = res.tile([8, 8], F32, tag="id8", name="id8")
        r["ones8"] = res.tile([B, 2 * 512], BF16, tag="ones8", name="ones8")
        r["selb"] = res.tile([128, NCORE * B], BF16, tag="selb", name="selb")
        r["hdecT"] = res.tile([128, KD * U], BF16, tag="hdecT", name="hdecT")
        r["c0"] = res.tile([128, B], F32, tag="c0", name="c0")
        r["c1"] = res.tile([128, B], F32, tag="c1", name="c1")
        # per-exchange buffers (k = 0..NEX; rxv[0] stays zero)
        r["rxv"] = [res.tile([128, NCORE * 16], BF16, tag=f"rxv{k}",
                             name=f"rxv{k}") for k in range(NEX + 1)]
        if fabric == "rdma":
            r["rxr"] = [res.tile([128, NCORE * 16], BF16, tag=f"rxr{k}",
                                 name=f"rxr{k}") for k in range(1, NEX + 1)]
        r["pay"] = [res.tile([128, 16], BF16, tag=f"pay{i}", name=f"pay{i}")
                    for i in range(2)]

        # ---------------- DMA resident weights ----------------
        nc.sync.dma_start(r["Whh0T"][:].rearrange("p (k n) -> p k n", n=GS),
                          D["Whh0T"].rearrange("(k p) n -> p k n", p=128))
        nc.sync.dma_start(r["Wih1T"][:].rearrange("p (k n) -> p k n", n=GS),
                          D["Wih1T"].rearrange("(k p) n -> p k n", p=128))
        nc.sync.dma_start(r["Whh1T"][:].rearrange("p (k n) -> p k n", n=GS),
                          D["Whh1T"].rearrange("(k p) n -> p k n", p=128))
        nc.sync.dma_start(r["wdecT"][:].rearrange("p (k n) -> p k n", n=ATT),
                          D["wdecT"].rearrange("(k p) n -> p k n", p=128))
        nc.sync.dma_start(r["wjdT"][:].rearrange("p (k n) -> p k n", n=JOINT),
                          D["wjdT"].rearrange("(k p) n -> p k n", p=128))
        nc.sync.dma_start(r["wjo"][:].rearrange("p (k n) -> p k n", n=ODIM),
                          D["wjo"].rearrange("(k p) n -> p k n", p=128))
        nc.sync.dma_start(r["WeT"][:].rearrange("p (k n) -> p k n", n=GS),
                          D["WeT"].rearrange("(k p) n -> p k n", p=128))
        for e in range(KEM):
            nc.sync.dma_start(r["eyT"][e][:], D["eyT"][e * 128:(e + 1) * 128, :])
        for nm in ["vatt", "batt", "bje", "idbf", "maskb", "id8", "ones8",
                   "selb", "b0_r", "b1_r"]:
            nc.sync.dma_start(r[nm][:], D[nm])
        nc.gpsimd.memset(r["c0"][:], 0.0)
        nc.gpsimd.memset(r["c1"][:], 0.0)
        nc.gpsimd.memset(r["hdecT"][:], 0.0)
        nc.gpsimd.memset(r["rxv"][0][:], 0.0)
        nc.gpsimd.memset(r["pay"][0][:], 0.0)
        nc.gpsimd.memset(r["pay"][1][:], 0.0)

        # ---------------- setup compute ----------------
        with tc.tile_pool(name="setup", bufs=1) as st:
            hsT = [st.tile([128, B * T], BF16, tag=f"hsT{k}", name=f"hsT{k}")
                   for k in range(KE)]
            for k in range(KE):
                nc.sync.dma_start(hsT[k][:], D["hsT"][k])
            hsTb = [st.tile([128, T], BF16, tag=f"hsTb{k}", name=f"hsTb{k}")
                    for k in range(KE)]
            for k in range(KE):
                nc.sync.dma_start(hsTb[k][:], D["hsTb"][k])
            wje = st.tile([128, KE * JOINT], BF16, tag="wje", name="wje")
            nc.sync.dma_start(wje[:].rearrange("p (k n) -> p k n", n=JOINT),
                              D["wje"].rearrange("(k p) n -> p k n", p=128))
            wenc = st.tile([128, KE * ATT], BF16, tag="wenc", name="wenc")
            nc.sync.dma_start(wenc[:].rearrange("p (k n) -> p k n", n=ATT),
                              D["wenc"].rearrange("(k p) n -> p k n", p=128))
            WaT = st.tile([128, KE * GS], BF16, tag="WaT", name="WaT")
            nc.sync.dma_start(WaT[:].rearrange("p (k n) -> p k n", n=GS),
                              D["WaT"].rearrange("(k p) n -> p k n", p=128))

            # preT[a][:, b*128+t] = (w_enc.T @ hsT)[a-block] + b_att
            for a in range(KA):
                for h in range(2):
                    pp = ps_big.tile([128, 512], F32, tag="psb", name="psb")
                    for k in range(KE):
                        nc.tensor.matmul(
                            pp[:], wenc[:, k * ATT + a * 128:k * ATT + (a + 1) * 128],
                            hsT[k][:, h * 512:(h + 1) * 512],
                            start=(k == 0), stop=(k == KE - 1))
                    nc.scalar.activation(r["preT"][a][:, h * 512:(h + 1) * 512],
                                         pp[:], AF.Identity,
                                         bias=r["batt"][:, a:a + 1])

            # P_sb[b][t, g] = hs[b] @ Wa_s.T
            for b in range(B):
                pp = ps_big.tile([128, GS], F32, tag="psb", name="psb")
                for k in range(KE):
                    nc.tensor.matmul(pp[:], hsT[k][:, b * 128:(b + 1) * 128],
                                     WaT[:, k * GS:(k + 1) * GS],
                                     start=(k == 0), stop=(k == KE - 1))
                nc.vector.tensor_copy(r["P_sb"][b][:], pp[:])

            # jeT[j] = (w_je.T @ hs[own].T)[j-block] + b_je
            for j in range(KJ):
                pp = ps_sm.tile([128, 128], F32, tag="sm", name="ps_je")
                for k in range(KE):
                    nc.tensor.matmul(
                        pp[:], wje[:, k * JOINT + j * 128:k * JOINT + (j + 1) * 128],
                        hsTb[k][:], start=(k == 0), stop=(k == KE - 1))
                nc.scalar.activation(r["jeT"][j][:], pp[:], AF.Identity,
                                     bias=r["bje"][:, j:j + 1])

            if fabric == "rdma":
                nc.gpsimd.sem_clear(sems["rsem"])
                nc.gpsimd.sem_clear(sems["lsem"])
                nc.gpsimd.sem_clear(sems["psem"])
                bar_in = dram.tile([128, 1], F32, tag="bar_in", name="bar_in")
                bar_out = dram.tile([NCORE * 128, 1], F32, tag="bar_out",
                                    name="bar_out", addr_space="Shared")
                nc.sync.dma_start(bar_in[:], D["id8"][0:128 // 128, 0:1]
                                  if False else D["maskb"][0:1, 0:1]
                                  .broadcast_to([128, 1]))
                nc.gpsimd.collective_compute(
                    "AllGather", ALU.bypass, ins=[bar_in[:].opt()],
                    outs=[bar_out[:].opt()],
                    replica_groups=[list(range(NCORE))])

        # ---------------- main loop ----------------
        with (
            tc.tile_pool(name="mp", bufs=2) as mp,
            tc.tile_pool(name="tanhp", bufs=2) as tanhp,
            tc.tile_pool(name="ztp", bufs=2) as ztp,
        ):
            env = dict(r=r, mp=mp, tanhp=tanhp, ztp=ztp, ps_big=ps_big,
                       ps_g=ps_g, ps_q=ps_q, ps_sm=ps_sm, dram=dram, D=D,
                       fabric=fabric, sems=sems, post_waits=post_waits)
            for rep in range(repeat):
                _main_rep(nc, tc, env, rep)


def _rx_h0(rxv, kd):
    return rxv[:, kd * 16:kd * 16 + 8]


def _rx_h1(rxv, kd):
    return rxv[:, kd * 16 + 8:kd * 16 + 16]


def _main_rep(nc, tc, env, rep):
    r, D = env["r"], env["D"]
    mp, tanhp, ztp = env["mp"], env["tanhp"], env["ztp"]
    ps_big, ps_g, ps_q, ps_sm = (env["ps_big"], env["ps_g"], env["ps_q"],
                                 env["ps_sm"])
    fabric, sems, dram = env["fabric"], env["sems"], env["dram"]
    post_waits = env["post_waits"]
    ex_base = rep * NEX

    ag_out_prev = None
    jdT_sb = [None, None]

    for k in range(NEX):
        ex = ex_base + k
        pay = r["pay"][k % 2]
        # early descgen for this step's exchange (reads only addresses; the
        # RAW on pay is against the write from two exchanges ago)
        if fabric == "rdma":
            preps = []
            for j in range(1, NCORE):
                rdests = [None] * NCORE
                rdests[j] = (0, j)
                p = nc.gpsimd.remote_dma_broadcast(
                    out_ap=r["rxr"][k + 1][:, j * 16:(j + 1) * 16],
                    in_ap=pay[:], remote_sem=sems["rsem"],
                    local_sem=sems["lsem"], rdests=rdests)
                p.then_inc(sems["psem"], 1)
                preps.append(p)
            # WAR: this step's payload writes must wait until the send from
            # exchange k-2 (same buffer) has fully left SBUF
            paycar = nc.vector.nop(hint=f"payw{rep}_{k}", nofuse=True)
            if ex >= 2:
                post_waits.append((paycar, sems["lsem"], 112 * (ex - 1)))
        else:
            paycar = None
        # ---- 1. obtain rxv[k] ----
        if k > 0:
            if fabric == "ag":
                nc.sync.dma_start(
                    r["rxv"][k][:].rearrange("p (g x) -> p g x", x=16),
                    ag_out_prev[:].rearrange("(g p) x -> p g x", p=128))
            else:
                carrier = nc.vector.nop(hint=f"rxw{rep}_{k}", nofuse=True)
                gate = nc.vector.tensor_copy(r["rxv"][k][:], r["rxr"][k][:])
                add_dep_helper(gate.ins, carrier.ins, sync=False,
                               reason="gate after rsem wait")
                post_waits.append((carrier, sems["rsem"], 14 * (ex_base + k)))
        rxv = r["rxv"][k][:]

        last_step = (k == NEX - 1)

        # ---- 2-7. attention + L0 (not in the last, L1-only step) ----
        if not last_step:
            # qT [128 att, KA*8] then transpose per a-tile -> q_ba [8, 512]
            qtp = ps_g.tile([128, 32], F32, tag="g", name="qtp")
            for a in range(KA):
                o = qtp[:, a * 8:(a + 1) * 8]
                for kd in range(KD):
                    nc.tensor.matmul(
                        o, r["wdecT"][:, kd * ATT + a * 128:kd * ATT + (a + 1) * 128],
                        _rx_h0(rxv, kd), start=(kd == 0), stop=(kd == KD - 1))
            qt_sb = mp.tile([128, 32], BF16, tag="qt_sb", name="qt_sb")
            nc.vector.tensor_copy(qt_sb[:], qtp[:])
            qbp = ps_q.tile([B, ATT], BF16, tag="qba", name="qba")
            for a in range(KA):
                nc.tensor.transpose(qbp[:, a * 128:(a + 1) * 128],
                                    qt_sb[:, a * 8:(a + 1) * 8], r["idbf"][:])
            qba = mp.tile([B, ATT], BF16, tag="qba_sb", name="qba_sb")
            nc.vector.tensor_copy(qba[:], qbp[:])

            # tanh tiles: pt = preT[a] + q outer, tanh -> tanh_sb
            tanh_sb = [tanhp.tile([128, B * 128], BF16, tag=f"tanh{a}",
                                  name=f"tanh{a}") for a in range(KA)]
            for a in range(KA):
                for h in range(2):
                    pt = ps_big.tile([128, 512], F32, tag="psb", name="psb")
                    nc.tensor.matmul(pt[:], r["idbf"][:],
                                     r["preT"][a][:, h * 512:(h + 1) * 512],
                                     start=True, stop=False)
                    nc.tensor.matmul(pt[:], qba[:, a * 128:(a + 1) * 128],
                                     r["ones8"][:, h * 512:(h + 1) * 512],
                                     start=False, stop=True)
                    nc.scalar.activation(tanh_sb[a][:, h * 512:(h + 1) * 512],
                                         pt[:], AF.Tanh)

            # eT[t, b] = sum_a v_a tanh_a[b,t]  (psum [128, 8])
            eps = ps_sm.tile([128, B], F32, tag="sm", name="eT")
            for b in range(B):
                for a in range(KA):
                    nc.tensor.matmul(eps[:, b:b + 1],
                                     tanh_sb[a][:, b * 128:(b + 1) * 128],
                                     r["vatt"][:, a:a + 1],
                                     start=(a == 0), stop=(a == KA - 1))
            eT_sb = mp.tile([128, B], BF16, tag="eT_sb", name="eT_sb")
            nc.vector.tensor_copy(eT_sb[:], eps[:])
            e8p = ps_sm.tile([B, T], BF16, tag="sm", name="e8")
            nc.tensor.transpose(e8p[:], eT_sb[:], r["idbf"][:])
            e8m = mp.tile([B, T], F32, tag="e8m", name="e8m")
            nc.vector.tensor_tensor(out=e8m[:], in0=e8p[:], in1=r["maskb"][:],
                                    op=ALU.add)
            w8 = mp.tile([B, T], F32, tag="w8", name="w8")
            ssum = mp.tile([B, 1], F32, tag="ssum", name="ssum")
            nc.scalar.activation(w8[:], e8m[:], AF.Exp, accum_out=ssum[:])
            rs = mp.tile([B, 1], F32, tag="rs", name="rs")
            nc.vector.reciprocal(rs[:], ssum[:])
            w8n = mp.tile([B, T], BF16, tag="w8n", name="w8n")
            nc.vector.tensor_scalar_mul(w8n[:], w8[:], rs[:])
            wtp = ps_sm.tile([128, B], BF16, tag="sm", name="wt")
            nc.tensor.transpose(wtp[:], w8n[:], r["idbf"][0:8, 0:8])
            wt = mp.tile([128, B], BF16, tag="wt_sb", name="wt_sb")
            nc.vector.tensor_copy(wt[:], wtp[:])

            # L0 gates psum [128, 4*8]
            g0 = ps_g.tile([128, 32], F32, tag="g", name="g0")
            for gt in range(4):
                o = g0[:, gt * 8:(gt + 1) * 8]
                for e in range(KEM):
                    nc.tensor.matmul(
                        o, r["WeT"][:, e * GS + gt * 128:e * GS + (gt + 1) * 128],
                        r["eyT"][e][:, k * 8:(k + 1) * 8],
                        start=(e == 0), stop=False)
                for kd in range(KD):
                    nc.tensor.matmul(
                        o, r["Whh0T"][:, kd * GS + gt * 128:kd * GS + (gt + 1) * 128],
                        _rx_h0(rxv, kd), start=False, stop=False)
                for b in range(B):
                    nc.tensor.matmul(
                        g0[:, gt * 8 + b:gt * 8 + b + 1],
                        r["P_sb"][b][:, gt * 128:(gt + 1) * 128],
                        wt[:, b:b + 1], start=False, stop=(b == B - 1),
                        skip_group_check=True)
            hw0 = _lstm_tail_T(nc, mp, g0, r["b0_r"], r["c0"],
                               pay[:, 0:8], "t0")
            if paycar is not None:
                add_dep_helper(hw0.ins, paycar.ins, sync=False,
                               reason="pay WAR vs in-flight send")

        # ---- 8. L1 for step k-1 ----
        if k > 0:
            g1 = ps_g.tile([128, 32], F32, tag="g", name="g1")
            for gt in range(4):
                o = g1[:, gt * 8:(gt + 1) * 8]
                for kd in range(KD):
                    nc.tensor.matmul(
                        o, r["Wih1T"][:, kd * GS + gt * 128:kd * GS + (gt + 1) * 128],
                        _rx_h0(rxv, kd), start=(kd == 0), stop=False)
                for kd in range(KD):
                    nc.tensor.matmul(
                        o, r["Whh1T"][:, kd * GS + gt * 128:kd * GS + (gt + 1) * 128],
                        _rx_h1(rxv, kd), start=False,
                        stop=(kd == KD - 1))
            hw1 = _lstm_tail_T(nc, mp, g1, r["b1_r"], r["c1"],
                               pay[:, 8:16], "t1")
            if paycar is not None:
                add_dep_helper(hw1.ins, paycar.ins, sync=False,
                               reason="pay WAR vs in-flight send")

        # ---- 13. send exchange k (emitted before the joint work so the
        # joint matmuls execute inside the collective's window) ----
        if fabric == "ag":
            ag_in = dram.tile([128, 16], BF16, tag="ag_in", name="ag_in")
            ag_out = dram.tile([NCORE * 128, 16], BF16, tag="ag_out",
                               name="ag_out", addr_space="Shared")
            fence = nc.sync.dma_start(ag_in[:], pay[:])
            nc.gpsimd.collective_compute(
                "AllGather", ALU.bypass, ins=[ag_in[:].opt()],
                outs=[ag_out[:].opt()], replica_groups=[list(range(NCORE))])
            ag_out_prev = ag_out
        else:
            cs = nc.vector.tensor_copy(r["rxr"][k + 1][:, 0:16], pay[:])
            trig = nc.gpsimd.trigger_dma(count=7)
            post_waits.append((trig, sems["psem"], 7 * (ex + 1)))
            add_dep_helper(trig.ins, cs.ins, sync=True,
                           reason="payload+selfcopy before trigger")
            fence = trig

        # ---- 10. hdecT col k-2 from rxv[k] ----
        if k >= 2:
            _hdec_col(nc, mp, r, rxv, k - 2)

        # ---- 11. jd for u-block ----
        if k >= 5 and (k - 5) % 4 == 0:
            j = (k - 5) // 4
            jdT_sb[j % 2] = _jd_block(nc, mp, ps_g, r, j)

        # ---- 12. joint for u = k-5, fenced behind the send so it fills
        # the collective's 15.8us window instead of the compute window ----
        if k >= 5:
            u = k - 5
            _joint_u(nc, ztp, ps_big, r, D, jdT_sb[(u // 4) % 2], u,
                     fence=fence)

    # ---- epilogue: last exchange -> hdecT col U-1, final joint ----
    k = NEX  # rxv index for the final exchange result
    if fabric == "ag":
        nc.sync.dma_start(
            r["rxv"][k][:].rearrange("p (g x) -> p g x", x=16),
            ag_out_prev[:].rearrange("(g p) x -> p g x", p=128))
    else:
        carrier = nc.vector.nop(hint=f"rxw{rep}_fin", nofuse=True)
        gate = nc.vector.tensor_copy(r["rxv"][k][:], r["rxr"][k][:])
        add_dep_helper(gate.ins, carrier.ins, sync=False,
                       reason="gate after rsem wait")
        post_waits.append((carrier, sems["rsem"], 14 * (ex_base + k)))
    _hdec_col(nc, mp, r, r["rxv"][k][:], U - 1)
    jdT_sb[(U // 4 - 1) % 2] = _jd_block(nc, mp, ps_g, r, U // 4 - 1)
    for u in range(U - 4, U):
        _joint_u(nc, ztp, ps_big, r, D, jdT_sb[(u // 4) % 2], u)


def _lstm_tail_T(nc, mp, g_ps, b_r, c_state, pay_slice, tag):
    """Transposed LSTM tail: g_ps [128 units, 4*8 (gate,b)] -> h bf16 to pay.

    sigmoid(x) is computed as 0.5*tanh(x/2)+0.5 so the ACT engine never
    leaves the exp_and_others table set (avoids 1.28us table reloads)."""
    sig = mp.tile([128, 32], F32, tag=f"{tag}_sig", name=f"{tag}_sig")
    nc.scalar.activation(sig[:, 0:8], g_ps[:, 0:8], AF.Tanh,
                         bias=b_r[:, 0:1], scale=0.5)
    nc.scalar.activation(sig[:, 8:16], g_ps[:, 8:16], AF.Tanh,
                         bias=b_r[:, 1:2], scale=0.5)
    nc.scalar.activation(sig[:, 16:24], g_ps[:, 16:24], AF.Tanh,
                         bias=b_r[:, 2:3])
    nc.scalar.activation(sig[:, 24:32], g_ps[:, 24:32], AF.Tanh,
                         bias=b_r[:, 3:4], scale=0.5)
    for c0_, c1_ in ((0, 8), (8, 16), (24, 32)):
        nc.vector.tensor_scalar(sig[:, c0_:c1_], sig[:, c0_:c1_], 0.5, 0.5,
                                op0=ALU.mult, op1=ALU.add)
    t1 = mp.tile([128, B], F32, tag=f"{tag}_t1", name=f"{tag}_t1")
    nc.vector.tensor_tensor(out=t1[:], in0=sig[:, 8:16], in1=c_state[:],
                            op=ALU.mult)
    t2 = mp.tile([128, B], F32, tag=f"{tag}_t2", name=f"{tag}_t2")
    nc.vector.tensor_tensor(out=t2[:], in0=sig[:, 0:8], in1=sig[:, 16:24],
                            op=ALU.mult)
    nc.vector.tensor_tensor(out=c_state[:], in0=t1[:], in1=t2[:], op=ALU.add)
    tc_ = mp.tile([128, B], F32, tag=f"{tag}_tc", name=f"{tag}_tc")
    nc.scalar.activation(tc_[:], c_state[:], AF.Tanh)
    return nc.vector.tensor_tensor(out=pay_slice, in0=sig[:, 24:32],
                                   in1=tc_[:], op=ALU.mult)


def _hdec_col(nc, mp, r, rxv, u):
    hsel = mp.tile([128, NCORE * B], F32, tag="hsel", name="hsel")
    nc.vector.tensor_tensor(
        out=hsel[:].rearrange("p (s x) -> p s x", x=8),
        in0=rxv.rearrange("p (s x) -> p s x", x=16)[:, :, 8:16],
        in1=r["selb"][:].rearrange("p (s x) -> p s x", x=8), op=ALU.mult)
    hred = mp.tile([128, KD], F32, tag="hred", name="hred")
    nc.vector.tensor_reduce(
        out=hred[:].rearrange("p (g o) -> p g o", o=1),
        in_=hsel[:].rearrange("p (g j) -> p g j", j=B),
        op=ALU.add, axis=mybir.AxisListType.X)
    nc.vector.tensor_copy(
        r["hdecT"][:].rearrange("p (g u) -> p g u", u=U)[:, :, u:u + 1],
        hred[:].rearrange("p (g o) -> p g o", o=1))


def _jd_block(nc, mp, ps_g, r, j):
    """jdT for u-block j: [128 joint-units, KJ*4] sbuf f32."""
    u0 = j * 4
    jp = ps_g.tile([128, 32], F32, tag="g", name="jdp")
    for nj in range(KJ):
        o = jp[:, nj * 4:(nj + 1) * 4]
        for kd in range(KD):
            nc.tensor.matmul(
                o, r["wjdT"][:, kd * JOINT + nj * 128:kd * JOINT + (nj + 1) * 128],
                r["hdecT"][:, kd * U + u0:kd * U + u0 + 4],
                start=(kd == 0), stop=(kd == KD - 1))
    jdT = mp.tile([128, 32], F32, tag="jdT", name="jdT", bufs=2)
    nc.vector.tensor_copy(jdT[:], jp[:])
    return jdT


def _joint_u(nc, ztp, ps_big, r, D, jdT, u, fence=None):
    uu = u % 4
    zt = [ztp.tile([128, 128], BF16, tag=f"zt{j2}", name=f"zt{j2}", bufs=3)
          for j2 in range(KJ)]
    for j2 in range(KJ):
        a = nc.scalar.activation(zt[j2][:], r["jeT"][j2][:], AF.Tanh,
                                 bias=jdT[:, j2 * 4 + uu:j2 * 4 + uu + 1])
        if fence is not None:
            add_dep_helper(a.ins, fence.ins, sync=False,
                           reason="joint fills collective window")
    for n in range(ODIM // 512):
        zb = ps_big.tile([128, 512], F32, tag="psb", name="psb")
        for kk in range(KJ):
            nc.tensor.matmul(
                zb[:], zt[kk][:],
                r["wjo"][:, kk * ODIM + n * 512:kk * ODIM + (n + 1) * 512],
                start=(kk == 0), stop=(kk == KJ - 1))
        ost = ztp.tile([128, 512], BF16, tag="ost", name="ost", bufs=4)
        nc.vector.tensor_copy(ost[:], zb[:])
        nc.sync.dma_start(
            D["z"][:, u * ODIM + n * 512:u * ODIM + (n + 1) * 512], ost[:])


# ---- SPMD runner (same as v1) ----

import jax
from jax.sharding import Mesh, PartitionSpec, NamedSharding
from jax.experimental.shard_map import shard_map

from concourse import bass2jax
from concourse.bass2jax import _bass_exec_p, partition_id_tensor


def build_spmd_fn(nc: bass.Bass, n_cores: int):
    bass2jax.install_neuronx_cc_hook()
    partition_name = nc.partition_id_tensor.name if nc.partition_id_tensor else None

    in_names, out_names, out_avals, zero_outs = [], [], [], []
    for alloc in nc.m.functions[0].allocations:
        if not isinstance(alloc, mybir.MemoryLocationSet):
            continue
        name = alloc.memorylocations[0].name
        if alloc.kind == "ExternalInput":
            if name != partition_name:
                in_names.append(name)
        elif alloc.kind == "ExternalOutput":
            out_names.append(name)
            shape = tuple(alloc.tensor_shape)
            dtype = mybir.dt.np(alloc.dtype)
            out_avals.append(jax.core.ShapedArray(shape, dtype))
            zero_outs.append(np.zeros(shape, dtype))
    n_params = len(in_names)
    n_outs = len(out_avals)
    all_in_names = list(in_names) + list(out_names)
    if partition_name is not None:
        all_in_names.append(partition_name)

    def _body(*args):
        operands = list(args)
        if partition_name is not None:
            operands.append(partition_id_tensor())
        outs = _bass_exec_p.bind(
            *operands, out_avals=tuple(out_avals),
            in_names=tuple(all_in_names), out_names=tuple(out_names),
            lowering_input_output_aliases=(),
            sim_require_finite=True, sim_require_nnan=True, nc=nc)
        return tuple(outs)

    devices = jax.devices()[:n_cores]
    mesh = Mesh(np.asarray(devices), ("core",))
    in_specs = (PartitionSpec("core"),) * (n_params + n_outs)
    out_specs = (PartitionSpec("core"),) * n_outs
    donate = tuple(range(n_params, n_params + n_outs))
    sharded = jax.jit(
        shard_map(_body, mesh=mesh, in_specs=in_specs, out_specs=out_specs,
                  check_rep=False),
        donate_argnums=donate, keep_unused=True)

    shard0 = NamedSharding(mesh, PartitionSpec("core"))

    def stage_inputs(in_maps):
        per_core = [[np.asarray(m[name]) for name in in_names] for m in in_maps]
        concat_in = [np.concatenate([per_core[c][i] for c in range(n_cores)], axis=0)
                     for i in range(n_params)]
        staged = [jax.device_put(a, shard0) for a in concat_in]
        jax.block_until_ready(staged)
        return staged

    def stage_zeros():
        z = [jax.device_put(np.zeros((n_cores * s.shape[0], *s.shape[1:]), s.dtype), shard0)
             for s in zero_outs]
        jax.block_until_ready(z)
        return z

    def exec_staged(staged_in, staged_zeros, return_outputs=True):
        out_arrs = sharded(*staged_in, *staged_zeros)
        jax.block_until_ready(out_arrs)
        if not return_outputs:
            return None
        return [
            {name: np.asarray(out_arrs[i]).reshape(n_cores, *out_avals[i].shape)[c]
             for i, name in enumerate(out_names)}
            for c in range(n_cores)
        ]

    def fn(in_maps, return_outputs=True):
        return exec_staged(stage_inputs(in_maps), stage_zeros(), return_outputs)

    fn.stage_inputs = stage_inputs
    fn.stage_zeros = stage_zeros
    fn.exec_staged = exec_staged
    return fn, in_names, out_names


_CACHED = None


def _get_fn():
    global _CACHED
    if _CACHED is None:
        nc = build_nc()
        fn, _, _ = build_spmd_fn(nc, NCORE)
        _CACHED = fn
    return _CACHED


def kernel(**inputs):
    per_core = host_prep(inputs)
    fn = _get_fn()
    res = fn(per_core)
    out = np.stack([res[c]["z"].reshape(T, U, ODIM) for c in range(NCORE)])
    return out.astype(np.float32)
